# revision 1
# baseline (speedup 1.0000x reference)
"""Hamilton-Adams demosaic kernel for Trainium2 (8 NeuronCores, data-parallel over batch).

Input:  x [8, 4, 768, 768] f32  (Bayer quarter-res planes: P00=R, P01=Gr, P10=Gb, P11=B)
Output: [8, 3, 1536, 1536] f32  (R, G, B full-res)

Phase-domain computation: every output colour plane is assembled from its 4 phase
sub-planes, so no mask multiplies are needed.

Compute-engine SBUF access must start at a 32-aligned partition, so vertical
(row-direction) stencil taps use DMA-shifted copies of the plane tiles (HBM re-reads
at +/-1 row; SBUF->SBUF shifts for the computed green planes).  All compute ops then
run on full 128-partition tiles with free-dim (column) AP offsets only.

Strips: 7 strips of 128 quarter-rows advancing by 124; lanes 2..125 of each strip are
shipped (lane p <-> quarter-row r0-2+p).  Plane tiles are [128, 770] with one halo
column each side holding mosaic edge-replication values.
"""
import sys
sys.path.insert(0, '/opt/trn_rl_repo')

from contextlib import ExitStack

import numpy as np

import concourse.bass as bass
import concourse.bacc as bacc
import concourse.tile as tile
from concourse import mybir
from concourse.bass_utils import run_bass_kernel_spmd

F = mybir.dt.float32
AL = mybir.AluOpType
AF = mybir.ActivationFunctionType

H = 768
PW = 770
NCORES = 8
STRIPS = [0, 124, 248, 372, 496, 620, 644]


class S:
    """Tiles + helpers for one strip."""

    def __init__(self, nc, pools, r0, first, last):
        self.nc = nc
        self.r0, self.first, self.last = r0, first, last
        pl, gr, tmp, out = pools
        mk = lambda n: pl.tile([128, PW], F, tag=n, name=n)
        self.p00, self.p01, self.p10, self.p11 = mk("p00"), mk("p01"), mk("p10"), mk("p11")
        # u = shifted up (lane p holds row+1), d = shifted down (row-1)
        self.p00u, self.p00d = mk("p00u"), mk("p00d")
        self.p10d, self.p01u = mk("p10d"), mk("p01u")
        self.p11u, self.p11d = mk("p11u"), mk("p11d")
        gk = lambda n: gr.tile([128, PW], F, tag=n, name=n)
        self.g00, self.g11, self.g00u, self.g11d = gk("g00"), gk("g11"), gk("g00u"), gk("g11d")
        if last:
            self.p00uz = pl.tile([128, PW], F, tag="pz", name="p00uz")
        if first:
            self.p11dz = pl.tile([128, PW], F, tag="pz", name="p11dz")
        ok = lambda n: out.tile([128, 3072], F, tag=n, name=n)
        self.out_r, self.out_g, self.out_b = ok("out_r"), ok("out_g"), ok("out_b")
        self._tmp = tmp

    def tmp(self):
        return self._tmp.tile([128, H], F, tag="tmp", name="tmp")

    def msk(self):
        return self._tmp.tile([128, H], mybir.dt.uint8, tag="msk", name="msk")

    @staticmethod
    def V(t, dj=0):  # interior view of padded tile, column shift dj
        return t[:, 1 + dj : 1 + dj + H]

    @staticmethod
    def OSL(t, a, b):  # strided output slice for phase (a, b), full partitions
        base = a * 1536 + b
        return t[:, base : min(base + 1536, 3072) : 2]


def _green_phase(s, cp, cpu, cpd, hp, ho, vp_a, vp_b, gdense):
    """Green at phase sites.  cp/cpu/cpd: centre plane + its up/down shifts.
    hp: horizontal neighbour plane, col offsets ho.  vp_a/vp_b: vertical neighbour
    plane tiles such that c1s = vp_a + vp_b (already the right shifted tiles)."""
    nc, V = s.nc, s.V
    c0s = s.tmp(); nc.vector.tensor_tensor(c0s[:], V(hp, ho[0]), V(hp, ho[1]), AL.add)
    c4 = s.tmp();  nc.vector.tensor_tensor(c4[:], V(hp, ho[0]), V(hp, ho[1]), AL.subtract)
    c1s = s.tmp(); nc.vector.tensor_tensor(c1s[:], V(vp_a), V(vp_b), AL.add)
    c5 = s.tmp();  nc.gpsimd.tensor_tensor(c5[:], V(vp_a), V(vp_b), AL.subtract)
    sh = s.tmp();  nc.vector.tensor_tensor(sh[:], V(cp, -1), V(cp, +1), AL.add)
    sv = s.tmp();  nc.gpsimd.tensor_tensor(sv[:], V(cpd), V(cpu), AL.add)
    c2 = s.tmp();  nc.vector.scalar_tensor_tensor(c2[:], V(cp), -2.0, sh[:], AL.mult, AL.add)
    c3 = s.tmp();  nc.vector.scalar_tensor_tensor(c3[:], V(cp), -2.0, sv[:], AL.mult, AL.add)
    b2 = s.tmp();  nc.vector.scalar_tensor_tensor(b2[:], c2[:], -0.5, c0s[:], AL.mult, AL.add)
    a2 = s.tmp();  nc.vector.scalar_tensor_tensor(a2[:], c3[:], -0.5, c1s[:], AL.mult, AL.add)
    q2 = s.tmp();  nc.scalar.activation(q2[:], c2[:], AF.Abs)
    q3 = s.tmp();  nc.scalar.activation(q3[:], c3[:], AF.Abs)
    q4 = s.tmp();  nc.scalar.activation(q4[:], c4[:], AF.Abs)
    q5 = s.tmp();  nc.scalar.activation(q5[:], c5[:], AF.Abs)
    clh = s.tmp(); nc.vector.tensor_tensor(clh[:], q4[:], q2[:], AL.add)
    clv = s.tmp(); nc.vector.tensor_tensor(clv[:], q5[:], q3[:], AL.add)
    d = s.tmp();   nc.vector.tensor_tensor(d[:], clh[:], clv[:], AL.subtract)
    m = s.msk();   nc.vector.tensor_scalar(m[:], d[:], 0.0, None, AL.is_gt)
    ah = s.tmp();  nc.scalar.mul(ah[:], a2[:], 0.5)
    nc.scalar.mul(V(gdense), b2[:], 0.5)
    nc.vector.copy_predicated(V(gdense), m[:], ah[:])


def _hv_field(s, xa, xb, ga, gb, co, addp, out_t, ab, efix=None):
    """out = 0.5*((xa@co0+xb@co1) - 0.5*(ga@co0+gb@co1) + addp) at phase ab.
    efix: 'R'/'L' - the masked-plane sum t1 sees a zero replication neighbour at the
    right/left image edge column; compute that column from the surviving operand."""
    nc, V = s.nc, s.V
    t1 = s.tmp()
    if efix == "R":
        nc.vector.tensor_tensor(t1[:, 0:767], xa[:, 1 + co[0] : 768 + co[0]], xb[:, 1 + co[1] : 768 + co[1]], AL.add)
        nc.vector.tensor_copy(t1[:, 767:768], xa[:, 768 + co[0] : 769 + co[0]])
    elif efix == "L":
        nc.vector.tensor_tensor(t1[:, 1:768], xa[:, 2 + co[0] : 769 + co[0]], xb[:, 2 + co[1] : 769 + co[1]], AL.add)
        nc.vector.tensor_copy(t1[:, 0:1], xb[:, 1 + co[1] : 2 + co[1]])
    else:
        nc.vector.tensor_tensor(t1[:], V(xa, co[0]), V(xb, co[1]), AL.add)
    t2 = s.tmp(); nc.vector.tensor_tensor(t2[:], V(ga, co[0]), V(gb, co[1]), AL.add)
    u = s.tmp();  nc.vector.scalar_tensor_tensor(u[:], t2[:], -0.5, t1[:], AL.mult, AL.add)
    v = s.tmp();  nc.vector.tensor_tensor(v[:], u[:], V(addp), AL.add)
    nc.scalar.mul(s.OSL(out_t, *ab), v[:], 0.5)


def _chan_blend(s, xp, xs, gf, gs, gnear, cP, cN, out_t, ab, efix=None):
    """R11/B00 diagonal interpolation.  xp: same-colour plane, xs: its vertical shift;
    gf: far green, gs: its shift; gnear: centre green.  cP/cN: column offsets
    (c_for_unshifted, c_for_shifted) of plus / minus diagonal pairs.  The shifted
    operand is the 2nd element of each diagonal pair."""
    nc, V = s.nc, s.V
    rp = s.tmp(); nc.vector.tensor_tensor(rp[:], V(xp, cP[0]), V(xs, cP[1]), AL.add)
    rn = s.tmp(); dn = s.tmp()
    if efix == "R":  # xp@cN0 hits zeroed right-edge replication at j=767
        nc.vector.tensor_tensor(rn[:, 0:767], xp[:, 1 + cN[0] : 768 + cN[0]], xs[:, 1 + cN[1] : 768 + cN[1]], AL.add)
        nc.vector.tensor_copy(rn[:, 767:768], xs[:, 768 + cN[1] : 769 + cN[1]])
        nc.gpsimd.tensor_tensor(dn[:, 0:767], xs[:, 1 + cN[1] : 768 + cN[1]], xp[:, 1 + cN[0] : 768 + cN[0]], AL.subtract)
        nc.vector.tensor_copy(dn[:, 767:768], xs[:, 768 + cN[1] : 769 + cN[1]])
    elif efix == "L":  # xs@cN1 hits zeroed left-edge replication at j=0
        nc.vector.tensor_tensor(rn[:, 1:768], xp[:, 2 + cN[0] : 769 + cN[0]], xs[:, 2 + cN[1] : 769 + cN[1]], AL.add)
        nc.vector.tensor_copy(rn[:, 0:1], xp[:, 1 + cN[0] : 2 + cN[0]])
        nc.gpsimd.tensor_tensor(dn[:, 1:768], xs[:, 2 + cN[1] : 769 + cN[1]], xp[:, 2 + cN[0] : 769 + cN[0]], AL.subtract)
        nc.vector.tensor_scalar(dn[:, 0:1], xp[:, 1 + cN[0] : 2 + cN[0]], -1.0, None, AL.mult)
    else:
        nc.vector.tensor_tensor(rn[:], V(xp, cN[0]), V(xs, cN[1]), AL.add)
        nc.gpsimd.tensor_tensor(dn[:], V(xs, cN[1]), V(xp, cN[0]), AL.subtract)
    dm = s.tmp(); nc.gpsimd.tensor_tensor(dm[:], V(xs, cP[1]), V(xp, cP[0]), AL.subtract)
    sp = s.tmp(); nc.gpsimd.tensor_tensor(sp[:], V(gf, cP[0]), V(gs, cP[1]), AL.add)
    sn = s.tmp(); nc.gpsimd.tensor_tensor(sn[:], V(gf, cN[0]), V(gs, cN[1]), AL.add)
    cg2 = s.tmp(); nc.vector.scalar_tensor_tensor(cg2[:], V(gnear), -2.0, sp[:], AL.mult, AL.add)
    cg3 = s.tmp(); nc.vector.scalar_tensor_tensor(cg3[:], V(gnear), -2.0, sn[:], AL.mult, AL.add)
    cp2 = s.tmp(); nc.vector.scalar_tensor_tensor(cp2[:], cg2[:], -0.5, rp[:], AL.mult, AL.add)
    cn2 = s.tmp(); nc.vector.scalar_tensor_tensor(cn2[:], cg3[:], -0.5, rn[:], AL.mult, AL.add)
    qp = s.tmp(); nc.scalar.activation(qp[:], cg2[:], AF.Abs)
    qn = s.tmp(); nc.scalar.activation(qn[:], cg3[:], AF.Abs)
    qdm = s.tmp(); nc.scalar.activation(qdm[:], dm[:], AF.Abs)
    qdn = s.tmp(); nc.scalar.activation(qdn[:], dn[:], AF.Abs)
    clp = s.tmp(); nc.vector.tensor_tensor(clp[:], qdm[:], qp[:], AL.add)
    cln = s.tmp(); nc.vector.tensor_tensor(cln[:], qdn[:], qn[:], AL.add)
    dr = s.tmp(); nc.vector.tensor_tensor(dr[:], clp[:], cln[:], AL.subtract)
    mr = s.msk(); nc.vector.tensor_scalar(mr[:], dr[:], 0.0, None, AL.is_gt)
    cnh = s.tmp(); nc.scalar.mul(cnh[:], cn2[:], 0.5)
    osl = s.OSL(out_t, *ab)
    nc.scalar.mul(osl, cp2[:], 0.5)
    nc.vector.copy_predicated(osl, mr[:], cnh[:])


def _load_plane(nc, t, xc, lo, hi):
    """DMA plane rows [lo, hi) (clamped to [0,768)) into lanes so lane p = row lo+p."""
    clo, chi = max(lo, 0), min(hi, H)
    nc.sync.dma_start(t[clo - lo : chi - lo, 1 : 1 + H], xc[clo:chi, :])


def _build_strip(nc, s, x, out_v):
    r0, first, last = s.r0, s.first, s.last
    V = s.V
    base = r0 - 2  # lane p <-> row base+p

    loads = [
        (s.p00, 0, 0), (s.p01, 1, 0), (s.p10, 2, 0), (s.p11, 3, 0),
        (s.p00u, 0, +1), (s.p00d, 0, -1), (s.p10d, 2, -1),
        (s.p01u, 1, +1), (s.p11u, 3, +1), (s.p11d, 3, -1),
    ]
    for t, c, sh in loads:
        _load_plane(nc, t, x[c], base + sh, base + sh + 128)

    # --- vertical edge replication (virtual rows <0 / >767), tiny row DMAs ---
    # value of virtual plane rows: row -1 of even-parity planes (p00,p01) = P00/P01[0];
    # p10/p11 row -1 = P00/P01[0]; row 768 of p00/p10 = P10[767], of p01/p11 = P11[767].
    def fill(t, lane, c, row):
        nc.sync.dma_start(t[lane : lane + 1, 1 : 1 + H], x[c, row : row + 1, :])

    if first:
        # lanes holding rows -2/-1 (never shipped, but must be finite & correct where read)
        for t, c in ((s.p00, 0), (s.p01, 1), (s.p10, 0), (s.p11, 1)):
            fill(t, 0, c, 0); fill(t, 1, c, 0)
        for t, c in ((s.p00u, 0), (s.p01u, 1), (s.p11u, 1)):
            fill(t, 0, c, 0)  # lane 0 = row -1
        for t, c in ((s.p00d, 0), (s.p10d, 0), (s.p11d, 1)):
            fill(t, 0, c, 0); fill(t, 1, c, 0); fill(t, 2, c, 0)  # lanes 0..2 = rows -3..-1
    if last:
        # strip r0=644: base=642.  main tiles: lanes 126,127 = rows 768,769
        for t, c in ((s.p00, 2), (s.p01, 3), (s.p10, 2), (s.p11, 3)):
            fill(t, 126, c, 767); fill(t, 127, c, 767)
        for t, c in ((s.p00u, 2), (s.p01u, 3), (s.p11u, 3)):
            fill(t, 125, c, 767); fill(t, 126, c, 767); fill(t, 127, c, 767)
        for t, c in ((s.p00d, 2), (s.p10d, 2), (s.p11d, 3)):
            fill(t, 127, c, 767)

    # --- horizontal halo columns (mosaic col replication; cross-plane for odd cols) ---
    cc = nc.vector.tensor_copy
    cc(s.p00[:, 0:1], s.p00[:, 1:2])
    cc(s.p01[:, 0:1], s.p00[:, 1:2])
    cc(s.p11[:, 0:1], s.p10[:, 1:2])
    cc(s.p00[:, PW - 1 : PW], s.p01[:, PW - 2 : PW - 1])
    cc(s.p10[:, PW - 1 : PW], s.p11[:, PW - 2 : PW - 1])
    cc(s.p11[:, PW - 1 : PW], s.p11[:, PW - 2 : PW - 1])
    nc.vector.memset(s.p00u[:, PW - 1 : PW], 0.0)  # xc_r replication at right edge is 0
    nc.vector.memset(s.p11d[:, 0:1], 0.0)          # xc_b replication at left edge is 0

    # red/blue vertical shifts need zero (masked) replication at image top/bottom:
    # use zero-lane variants of p00u (last strip) / p11d (first strip)
    p00u_rb, p11d_rb = s.p00u, s.p11d
    if s.last:
        p00u_rb = s._tmp.parent_pool_hack if False else None
        p00u_rb = s.p00uz
        nc.vector.memset(p00u_rb[:], 0.0)
        _load_plane(nc, p00u_rb, x[0], base + 1, base + 1 + 125)
    if s.first:
        p11d_rb = s.p11dz
        nc.vector.memset(p11d_rb[:], 0.0)
        _load_plane(nc, p11d_rb, x[3], base - 1, base - 1 + 128)

    # --- green interpolation ---
    # phase 00: centre p00, horiz p01 (j-1, j), vert c1s = p10[i-1]+p10[i]
    _green_phase(s, s.p00, s.p00u, s.p00d, s.p01, (-1, 0), s.p10d, s.p10, s.g00)
    # phase 11: centre p11, horiz p10 (j, j+1), vert c1s = p01[i]+p01[i+1]
    _green_phase(s, s.p11, s.p11u, s.p11d, s.p10, (0, +1), s.p01, s.p01u, s.g11)

    cc(s.g00[:, PW - 1 : PW], s.p01[:, PW - 2 : PW - 1])  # G00[:,768] = P01[:,767]
    cc(s.g11[:, 0:1], s.p10[:, 1:2])                      # G11[:,-1] = P10[:,0]
    if first:  # green at virtual row -1 (lane 1): g11 = P01[0] (= p01 lane 1)
        nc.sync.dma_start(s.g11[1:2, :], s.p01[1:2, :])
    if last:   # green at virtual row 768 (lane 126): g00 = P10[767] (= p10 lane 126)
        nc.sync.dma_start(s.g00[126:127, :], s.p10[126:127, :])

    # shifted green tiles (SBUF->SBUF row shift)
    nc.sync.dma_start(s.g00u[0:127, :], s.g00[1:128, :])
    nc.sync.dma_start(s.g11d[1:128, :], s.g11[0:127, :])

    # --- green output ---
    sc = nc.scalar.copy
    sc(s.OSL(s.out_g, 0, 0), V(s.g00))
    sc(s.OSL(s.out_g, 1, 1), V(s.g11))
    sc(s.OSL(s.out_g, 0, 1), V(s.p01))
    sc(s.OSL(s.out_g, 1, 0), V(s.p10))

    # --- red ---
    sc(s.OSL(s.out_r, 0, 0), V(s.p00))
    _hv_field(s, s.p00, s.p00, s.g00, s.g00, (0, +1), s.p01, s.out_r, (0, 1), efix="R")
    _hv_field(s, s.p00, p00u_rb, s.g00, s.g00u, (0, 0), s.p10, s.out_r, (1, 0))
    # R11: P pair (0,0)+(+1,+1), N pair (0,+1)+(+1,0); shifted operand = (+1,*)
    _chan_blend(s, s.p00, p00u_rb, s.g00, s.g00u, s.g11, (0, +1), (+1, 0), s.out_r, (1, 1), efix="R")

    # --- blue ---
    sc(s.OSL(s.out_b, 1, 1), V(s.p11))
    _hv_field(s, s.p11, s.p11, s.g11, s.g11, (-1, 0), s.p10, s.out_b, (1, 0), efix="L")
    _hv_field(s, p11d_rb, s.p11, s.g11d, s.g11, (0, 0), s.p01, s.out_b, (0, 1))
    # B00: P pair p11d@(-1)+p11@(0): cm4 = p11@(0) - p11d@(-1); N pair p11d@(0)+p11@(-1)
    _chan_blend(s, p11d_rb, s.p11, s.g11d, s.g11, s.g00, (-1, 0), (0, -1), s.out_b, (0, 0), efix="L")

    # --- output DMA (lanes 2..125 <-> rows r0..r0+123; last strip ships 102..125) ---
    if last:
        p0, pn, row0 = 102, 24, 744
    else:
        p0, pn, row0 = 2, 124, r0
    for c, t in enumerate((s.out_r, s.out_g, s.out_b)):
        nc.sync.dma_start(out_v[c, row0 : row0 + pn, :], t[p0 : p0 + pn, :])


def build_nc():
    nc = bacc.Bacc("TRN2", target_bir_lowering=False, debug=False, num_devices=NCORES)
    x_in = nc.declare_dram_parameter("x", [4, H, H], F, isOutput=False)
    out = nc.declare_dram_parameter("out", [3, 2 * H, 2 * H], F, isOutput=True)
    out_v = out[:].rearrange("c (r two) w -> c r (two w)", two=2)

    with tile.TileContext(nc) as tc, ExitStack() as ctx:
        pl = ctx.enter_context(tc.tile_pool(name="planes", bufs=2))
        gr = ctx.enter_context(tc.tile_pool(name="greens", bufs=2))
        tmp = ctx.enter_context(tc.tile_pool(name="temps", bufs=8))
        outp = ctx.enter_context(tc.tile_pool(name="outs", bufs=2))
        for si, r0 in enumerate(STRIPS):
            s = S(nc, (pl, gr, tmp, outp), r0, si == 0, si == len(STRIPS) - 1)
            _build_strip(nc, s, x_in[:], out_v)
    nc.compile()
    return nc


_NC_CACHE = None


def kernel(x: np.ndarray) -> np.ndarray:
    global _NC_CACHE
    if _NC_CACHE is None:
        _NC_CACHE = build_nc()
    x = np.ascontiguousarray(x, dtype=np.float32)
    in_maps = [{"x": x[i]} for i in range(NCORES)]
    res = run_bass_kernel_spmd(_NC_CACHE, in_maps, list(range(NCORES)))
    return np.stack([res.results[i]["out"] for i in range(NCORES)], axis=0)



# revision 2
# speedup vs baseline: 1.0422x; 1.0422x over previous
"""Hamilton-Adams demosaic v2 for Trainium2 (8 cores, data-parallel).

Device computes 8 fp16 phase planes [R01,R10,R11,B00,B01,B10,G00,G11] from 8
host-prepared padded fp16 quarter planes; host handles plane prep (padding /
masking / dtype) and output interleave + identity phases (R00=x0, G01=x1,
G10=x2, B11=x3).

Layout: 7 row-strips of 128 lanes (lane p = qrow r0-2+p, ship lanes 2..125).
All compute is same-partition fp16 ops except cross-lane vertical taps, which
run on the PE as banded matmuls accumulating in PSUM.
"""
import sys
sys.path.insert(0, '/opt/trn_rl_repo')

from contextlib import ExitStack

import numpy as np

import concourse.bass as bass
import concourse.bacc as bacc
import concourse.tile as tile
from concourse import mybir
from concourse.bass_utils import run_bass_kernel_spmd

F16 = mybir.dt.float16
F32 = mybir.dt.float32
U8 = mybir.dt.uint8
U16 = mybir.dt.uint16
AL = mybir.AluOpType
AF = mybir.ActivationFunctionType

H = 768
W = 772            # padded plane width (col = qcol + 2)
NCORES = 8
STRIPS = [0, 124, 248, 372, 496, 620, 644]

# input plane slots (order in IT tile and xp array)
P00, P01, P10, P11, P00Z, P11Z, P00UZ, P11DZ = range(8)
# band matrix ids
BANDS = {}


def _band_defs():
    """band[k, m] = weight of moving lane k for output lane m."""
    def mk(taps):
        b = np.zeros((128, 128), np.float16)
        for dk, w in taps:
            for m in range(128):
                k = m + dk
                if 0 <= k < 128:
                    b[k, m] = w
        return b
    defs = {
        "tri_n05": [(-1, -0.5), (0, 1.0), (1, -0.5)],     # -0.5 * (1,-2,1)
        "duo_m": [(-1, 1.0), (0, 1.0)],
        "duo_p": [(0, 1.0), (1, 1.0)],
        "d5_m": [(-1, 1.0), (0, -1.0)],
        "d5_p": [(0, 1.0), (1, -1.0)],
        "I": [(0, 1.0)],
        "I05": [(0, 0.5)],
        "I_n05": [(0, -0.5)],
        "up1": [(1, 1.0)],
        "up1_n05": [(1, -0.5)],
        "dn1_n05": [(-1, -0.5)],
        "duo_p_05": [(0, 0.5), (1, 0.5)],
        "duo_m_05": [(-1, 0.5), (0, 0.5)],
        "duo_p_n025": [(0, -0.25), (1, -0.25)],
        "duo_m_n025": [(-1, -0.25), (0, -0.25)],
    }
    names = list(defs)
    arr = np.stack([mk(defs[n]) for n in names])  # [NB,128,128]
    return names, arr


BAND_NAMES, BAND_ARR = _band_defs()
NB = len(BAND_NAMES)


class Ctx:
    def __init__(self, nc, pools, bd):
        self.nc = nc
        self.it_pool, self.g_pool, self.ot_pool, self.tmp_pool, self.ps_pool = pools
        self.bd = bd  # band tile [128, NB*128]

    def band(self, name):
        i = BAND_NAMES.index(name)
        return self.bd[:, 128 * i : 128 * i + 128]

    def tmp(self):
        return self.tmp_pool.tile([128, H], F16, tag="tmp", name="tmp")

    def msk(self):
        return self.tmp_pool.tile([128, H], U16, tag="msk", name="msk")

    def psum(self, tag):
        # 1024 f32 = exactly 2 PSUM banks; only cols 0:768 are used, and the
        # matmul chunk split (0:512, 512:768) keeps each write within a bank.
        return self.ps_pool.tile([128, 1024], F32, tag=tag, name=tag)


def _mm_band(nc, dst, band, mov_tile, mov_off, start):
    """dst [128,768] psum += band.T @ mov (768 cols from mov_tile at mov_off)."""
    for c0, cw in ((0, 512), (512, 256)):
        nc.tensor.matmul(
            dst[:, c0 : c0 + cw],
            band,
            mov_tile[:, mov_off + c0 : mov_off + c0 + cw],
            start=start, stop=True,
        )


def pe_accum(cx, dst, terms, tag=None):
    """terms: list of (band_name, mov_tile, mov_off). Accumulates into dst psum."""
    nc = cx.nc
    for i, (bn, mt, mo) in enumerate(terms):
        _mm_band(nc, dst, cx.band(bn), mt, mo, start=(i == 0))


def _green_phases(cx, IT, G, phA, phB):
    """Emit both green phases interleaved so each engine always has ready work."""
    nc = cx.nc
    s = lambda k, dj=0: IT[:, 772 * k + 2 + dj : 772 * k + 2 + dj + H]

    pa, q3, pc, q5, ah, c0s, c4 = {}, {}, {}, {}, {}, {}, {}
    sh, c2, b2, q2, q4, clh, clv, m = {}, {}, {}, {}, {}, {}, {}, {}
    P = {0: phA, 1: phB}

    for i in (0, 1):
        pa[i] = cx.psum("psA")
        pe_accum(cx, pa[i], [("tri_n05", IT, 772 * P[i]["ctr"] + 2)])
    for i in (0, 1):
        q3[i] = cx.tmp(); nc.scalar.activation(q3[i][:], pa[i][:, :H], AF.Abs, 0.0, -2.0)
    for i in (0, 1):
        _mm_band(nc, pa[i], cx.band(P[i]["duo"]), IT, 772 * P[i]["vp"] + 2, start=False)
    for i in (0, 1):
        pc[i] = cx.psum("psB")
        pe_accum(cx, pc[i], [(P[i]["d5"], IT, 772 * P[i]["vp"] + 2)])
    for i in (0, 1):
        q5[i] = cx.tmp(); nc.scalar.activation(q5[i][:], pc[i][:, :H], AF.Abs)
    for i in (0, 1):
        ah[i] = cx.tmp(); nc.scalar.activation(ah[i][:], pa[i][:, :H], AF.Copy, 0.0, 0.5)
    for i in (0, 1):
        o0, o1 = P[i]["ho"]; hp = P[i]["hp"]
        c0s[i] = cx.tmp(); nc.gpsimd.tensor_tensor(c0s[i][:], s(hp, o0), s(hp, o1), AL.add)
    for i in (0, 1):
        o0, o1 = P[i]["ho"]; hp = P[i]["hp"]
        c4[i] = cx.tmp(); nc.gpsimd.tensor_tensor(c4[i][:], s(hp, o0), s(hp, o1), AL.subtract)
    for i in (0, 1):
        ctr = P[i]["ctr"]
        sh[i] = cx.tmp(); nc.vector.tensor_tensor(sh[i][:], s(ctr, -1), s(ctr, +1), AL.add)
    for i in (0, 1):
        c2[i] = cx.tmp(); nc.vector.scalar_tensor_tensor(c2[i][:], s(P[i]["ctr"]), -2.0, sh[i][:], AL.mult, AL.add)
    for i in (0, 1):
        q2[i] = cx.tmp(); nc.vector.tensor_scalar(q2[i][:].bitcast(U16), c2[i][:].bitcast(U16), 0x7FFF, None, AL.bitwise_and)
    for i in (0, 1):
        q4[i] = cx.tmp(); nc.vector.tensor_scalar(q4[i][:].bitcast(U16), c4[i][:].bitcast(U16), 0x7FFF, None, AL.bitwise_and)
    for i in (0, 1):
        b2[i] = cx.tmp(); nc.vector.scalar_tensor_tensor(b2[i][:], c2[i][:], -0.5, c0s[i][:], AL.mult, AL.add)
    for i in (0, 1):
        clh[i] = cx.tmp(); nc.vector.tensor_tensor(clh[i][:], q2[i][:], q4[i][:], AL.add)
    for i in (0, 1):
        clv[i] = cx.tmp(); nc.vector.tensor_tensor(clv[i][:], q3[i][:], q5[i][:], AL.add)
    for i in (0, 1):
        m[i] = cx.msk(); nc.vector.tensor_tensor(m[i][:], clh[i][:], clv[i][:], AL.is_gt)
    for i in (0, 1):
        gslot = P[i]["gslot"]
        gd = G[:, 772 * gslot + 2 : 772 * gslot + 2 + H]
        nc.vector.tensor_scalar(gd, b2[i][:], 0.5, None, AL.mult)
        nc.vector.copy_predicated(gd, m[i][:], ah[i][:])


def _chroma(cx, IT, G, OT, hvH, hvV, cbs):
    """Emit hv fields + chan blends with A/B interleaving."""
    nc = cx.nc
    s = lambda k, dj=0: IT[:, 772 * k + 2 + dj : 772 * k + 2 + dj + H]
    gs = lambda k, dj=0: G[:, 772 * k + 2 + dj : 772 * k + 2 + dj + H]

    # hv vertical via PE
    pv = {}
    for i, (xz, g, addp, duoX, duoG, oslot) in enumerate(hvV):
        pv[i] = cx.psum("psB")
        pe_accum(cx, pv[i], [
            (duoX, IT, 772 * xz + 2),
            (duoG, G, 772 * g + 2),
            ("I05", IT, 772 * addp + 2),
        ])
    for i, (xz, g, addp, duoX, duoG, oslot) in enumerate(hvV):
        nc.scalar.activation(OT[:, H * oslot : H * oslot + H], pv[i][:, :H], AF.Copy)

    # cb pool diffs
    dm, dn = {}, {}
    for i, ch in enumerate(cbs):
        cP, cN = ch["cP"], ch["cN"]
        dm[i] = cx.tmp(); nc.gpsimd.tensor_tensor(dm[i][:], s(ch["xs"], cP[1]), s(ch["xp"], cP[0]), AL.subtract)
    for i, ch in enumerate(cbs):
        cP, cN = ch["cP"], ch["cN"]
        dn[i] = cx.tmp(); nc.gpsimd.tensor_tensor(dn[i][:], s(ch["xs"], cN[1]), s(ch["xp"], cN[0]), AL.subtract)

    # cb PE banks (p then n), with scalar abs between accumulations
    pp, pn, qp, qn = {}, {}, {}, {}
    for i, ch in enumerate(cbs):
        pp[i] = cx.psum("psA")
        pe_accum(cx, pp[i], [
            ("I", G, 772 * ch["gnear"] + 2),
            (ch["gP0"][0], G, 772 * ch["gf"] + 2 + ch["gP0"][1]),
            (ch["gP1"][0], G, 772 * ch["gf"] + 2 + ch["gP1"][1]),
        ])
    for i, ch in enumerate(cbs):
        qp[i] = cx.tmp(); nc.scalar.activation(qp[i][:], pp[i][:, :H], AF.Abs, 0.0, -2.0)
    for i, ch in enumerate(cbs):
        _mm_band(nc, pp[i], cx.band("I"), IT, 772 * ch["xp"] + 2 + ch["cP"][0], start=False)
        _mm_band(nc, pp[i], cx.band("I"), IT, 772 * ch["xs"] + 2 + ch["cP"][1], start=False)

    # hv horizontal on DVE (independent work while PE/scalar chew on cb)
    t1, t2, u = {}, {}, {}
    for i, (xz, xo, g, go, addp, oslot) in enumerate(hvH):
        t1[i] = cx.tmp(); nc.vector.tensor_tensor(t1[i][:], s(xz, xo[0]), s(xz, xo[1]), AL.add)
    for i, (xz, xo, g, go, addp, oslot) in enumerate(hvH):
        t2[i] = cx.tmp(); nc.vector.tensor_tensor(t2[i][:], gs(g, go[0]), gs(g, go[1]), AL.add)
    for i, (xz, xo, g, go, addp, oslot) in enumerate(hvH):
        u[i] = cx.tmp(); nc.vector.scalar_tensor_tensor(u[i][:], t2[i][:], -0.5, t1[i][:], AL.mult, AL.add)
    for i, (xz, xo, g, go, addp, oslot) in enumerate(hvH):
        v = cx.tmp(); nc.vector.tensor_tensor(v[:], u[i][:], s(addp), AL.add)
        nc.vector.tensor_scalar(OT[:, H * oslot : H * oslot + H], v[:], 0.5, None, AL.mult)

    for i, ch in enumerate(cbs):
        pn[i] = cx.psum("psB")
        pe_accum(cx, pn[i], [
            ("I", G, 772 * ch["gnear"] + 2),
            (ch["gN0"][0], G, 772 * ch["gf"] + 2 + ch["gN0"][1]),
            (ch["gN1"][0], G, 772 * ch["gf"] + 2 + ch["gN1"][1]),
        ])
    for i, ch in enumerate(cbs):
        qn[i] = cx.tmp(); nc.scalar.activation(qn[i][:], pn[i][:, :H], AF.Abs, 0.0, -2.0)
    for i, ch in enumerate(cbs):
        _mm_band(nc, pn[i], cx.band("I"), IT, 772 * ch["xp"] + 2 + ch["cN"][0], start=False)
        _mm_band(nc, pn[i], cx.band("I"), IT, 772 * ch["xs"] + 2 + ch["cN"][1], start=False)

    # classifiers + select
    qdm, qdn, clp, cln, m2, cnh = {}, {}, {}, {}, {}, {}
    for i in range(len(cbs)):
        qdm[i] = cx.tmp(); nc.vector.tensor_scalar(qdm[i][:].bitcast(U16), dm[i][:].bitcast(U16), 0x7FFF, None, AL.bitwise_and)
    for i in range(len(cbs)):
        qdn[i] = cx.tmp(); nc.vector.tensor_scalar(qdn[i][:].bitcast(U16), dn[i][:].bitcast(U16), 0x7FFF, None, AL.bitwise_and)
    for i in range(len(cbs)):
        clp[i] = cx.tmp(); nc.vector.tensor_tensor(clp[i][:], qdm[i][:], qp[i][:], AL.add)
    for i in range(len(cbs)):
        cln[i] = cx.tmp(); nc.vector.tensor_tensor(cln[i][:], qdn[i][:], qn[i][:], AL.add)
    for i in range(len(cbs)):
        m2[i] = cx.msk(); nc.vector.tensor_tensor(m2[i][:], clp[i][:], cln[i][:], AL.is_gt)
    for i, ch in enumerate(cbs):
        osl = OT[:, H * ch["oslot"] : H * ch["oslot"] + H]
        nc.scalar.activation(osl, pp[i][:, :H], AF.Copy, 0.0, 0.5)
    for i, ch in enumerate(cbs):
        cnh[i] = cx.tmp(); nc.scalar.activation(cnh[i][:], pn[i][:, :H], AF.Copy, 0.0, 0.5)
    for i, ch in enumerate(cbs):
        osl = OT[:, H * ch["oslot"] : H * ch["oslot"] + H]
        nc.vector.copy_predicated(osl, m2[i][:], cnh[i][:])


# output plane order in device out tensor
OR01, OR10, OR11, OB00, OB01, OB10 = range(6)  # OT slots; G00, G11 from G tile
G00S, G11S = 0, 1


def _build_strip(cx, IT, G, OT, xp_v, out_v, r0, first, last):
    nc = cx.nc

    # input DMA: 4 instructions (2 slots each); partition dim outermost on SBUF side
    it_v = IT[:].rearrange("p (k c) -> p k c", k=8)
    xr = xp_v.rearrange("k r c -> r k c")
    for k0 in range(0, 8, 2):
        nc.sync.dma_start(it_v[:, k0 : k0 + 2, :], xr[r0 : r0 + 128, k0 : k0 + 2, :])

    _green_phases(cx, IT, G,
                  dict(ctr=P00, hp=P01, ho=(-1, 0), vp=P10, duo="duo_m", d5="d5_m", gslot=G00S),
                  dict(ctr=P11, hp=P10, ho=(0, +1), vp=P01, duo="duo_p", d5="d5_p", gslot=G11S))

    # green halo cols: g00[:,770] = p01 col 767 ; g11[:,1] = p10 col 0
    nc.vector.tensor_copy(G[:, 770:771], IT[:, 772 * P01 + 769 : 772 * P01 + 770])
    nc.vector.tensor_copy(G[:, 772 + 1 : 772 + 2], IT[:, 772 * P10 + 2 : 772 * P10 + 3])
    if first:
        nc.sync.dma_start(G[1:2, 772:1544], IT[1:2, 772 * P01 : 772 * P01 + 772])
    if last:
        nc.sync.dma_start(G[126:127, 0:772], IT[126:127, 772 * P10 : 772 * P10 + 772])

    _chroma(cx, IT, G, OT,
            hvH=[(P00Z, (0, +1), G00S, (0, +1), P01, OR01),
                 (P11Z, (-1, 0), G11S, (-1, 0), P10, OB10)],
            hvV=[(P00Z, G00S, P10, "duo_p_05", "duo_p_n025", OR10),
                 (P11Z, G11S, P01, "duo_m_05", "duo_m_n025", OB01)],
            cbs=[dict(xp=P00Z, xs=P00UZ, cP=(0, +1), cN=(+1, 0), gf=G00S, gnear=G11S,
                      gP0=("I_n05", 0), gP1=("up1_n05", +1),
                      gN0=("I_n05", +1), gN1=("up1_n05", 0), oslot=OR11),
                 dict(xp=P11DZ, xs=P11Z, cP=(-1, 0), cN=(0, -1), gf=G11S, gnear=G00S,
                      gP0=("dn1_n05", -1), gP1=("I_n05", 0),
                      gN0=("dn1_n05", 0), gN1=("I_n05", -1), oslot=OB00)])

    # output DMA
    if last:
        p0, pn_, row0 = 102, 24, 744
    else:
        p0, pn_, row0 = 2, 124, r0
    ot_v = OT[:].rearrange("p (k c) -> p k c", k=6)
    our = out_v.rearrange("k r c -> r k c")
    nc.sync.dma_start(our[row0 : row0 + pn_, 0:3, :], ot_v[p0 : p0 + pn_, 0:3, :])
    nc.sync.dma_start(our[row0 : row0 + pn_, 3:6, :], ot_v[p0 : p0 + pn_, 3:6, :])
    g_int = G[:].rearrange("p (k c) -> p k c", k=2)[:, :, 2 : 2 + H]
    nc.sync.dma_start(our[row0 : row0 + pn_, 6:8, :], g_int[p0 : p0 + pn_, :, :])


def build_nc():
    nc = bacc.Bacc("TRN2", target_bir_lowering=False, debug=False, num_devices=NCORES)
    xp_in = nc.declare_dram_parameter("xp", [8, W, W], F16, isOutput=False)
    bd_in = nc.declare_dram_parameter("bands", [128, NB * 128], F16, isOutput=False)
    out = nc.declare_dram_parameter("out", [8, H, H], F16, isOutput=True)

    with tile.TileContext(nc) as tc, ExitStack() as ctx:
        itp = ctx.enter_context(tc.tile_pool(name="it", bufs=3))
        gp = ctx.enter_context(tc.tile_pool(name="g", bufs=2))
        otp = ctx.enter_context(tc.tile_pool(name="ot", bufs=2))
        tmp = ctx.enter_context(tc.tile_pool(name="tmp", bufs=16))
        psp = ctx.enter_context(tc.tile_pool(name="ps", bufs=2, space="PSUM"))
        bdp = ctx.enter_context(tc.tile_pool(name="bd", bufs=1))

        bd = bdp.tile([128, NB * 128], F16, name="bands")
        nc.sync.dma_start(bd[:], bd_in[:])

        for si, r0 in enumerate(STRIPS):
            IT = itp.tile([128, 8 * W], F16, tag="IT", name="IT")
            G = gp.tile([128, 2 * W], F16, tag="G", name="G")
            OT = otp.tile([128, 6 * H], F16, tag="OT", name="OT")
            cx = Ctx(nc, (itp, gp, otp, tmp, psp), bd)
            _build_strip(cx, IT, G, OT, xp_in[:], out[:], r0,
                         si == 0, si == len(STRIPS) - 1)
    nc.compile()
    return nc


def host_planes(P):
    """P: [4,768,768] f32. Returns [8,772,772] fp16 padded planes."""
    M = np.empty((1536, 1536), np.float32)
    M[0::2, 0::2] = P[0]
    M[0::2, 1::2] = P[1]
    M[1::2, 0::2] = P[2]
    M[1::2, 1::2] = P[3]
    Mzr = np.zeros_like(M); Mzr[0::2, 0::2] = P[0]
    Mzb = np.zeros_like(M); Mzb[1::2, 1::2] = P[3]
    q = np.arange(-2, 770)

    def mk(src, a, b, dr=0):
        ri = np.clip(2 * (q + dr) + a, 0, 1535)
        ci = np.clip(2 * q + b, 0, 1535)
        return src[np.ix_(ri, ci)]

    pl = np.empty((8, 772, 772), np.float16)
    pl[P00] = mk(M, 0, 0)
    pl[P01] = mk(M, 0, 1)
    pl[P10] = mk(M, 1, 0)
    pl[P11] = mk(M, 1, 1)
    pl[P00Z] = mk(Mzr, 0, 0)
    pl[P11Z] = mk(Mzb, 1, 1)
    pl[P00UZ] = mk(Mzr, 0, 0, dr=+1)
    pl[P11DZ] = mk(Mzb, 1, 1, dr=-1)
    return pl


_NC_CACHE = None


def kernel(x: np.ndarray) -> np.ndarray:
    global _NC_CACHE
    if _NC_CACHE is None:
        _NC_CACHE = build_nc()
    x = np.ascontiguousarray(x, dtype=np.float32)
    bands = np.ascontiguousarray(BAND_ARR.transpose(1, 0, 2).reshape(128, NB * 128))
    in_maps = [{"xp": host_planes(x[i]), "bands": bands} for i in range(NCORES)]
    res = run_bass_kernel_spmd(_NC_CACHE, in_maps, list(range(NCORES)))
    out = np.empty((NCORES, 3, 1536, 1536), np.float32)
    for i in range(NCORES):
        d = res.results[i]["out"].astype(np.float32)
        out[i, 0, 0::2, 0::2] = x[i, 0]
        out[i, 0, 0::2, 1::2] = d[OR01]
        out[i, 0, 1::2, 0::2] = d[OR10]
        out[i, 0, 1::2, 1::2] = d[OR11]
        out[i, 1, 0::2, 0::2] = d[6]
        out[i, 1, 0::2, 1::2] = x[i, 1]
        out[i, 1, 1::2, 0::2] = x[i, 2]
        out[i, 1, 1::2, 1::2] = d[7]
        out[i, 2, 0::2, 0::2] = d[OB00]
        out[i, 2, 0::2, 1::2] = d[OB01]
        out[i, 2, 1::2, 0::2] = d[OB10]
        out[i, 2, 1::2, 1::2] = x[i, 3]
    return out


def make_trace_inmaps(x):
    """in_maps for a traced run (test harness helper)."""
    x = np.ascontiguousarray(x, dtype=np.float32)
    bands = np.ascontiguousarray(BAND_ARR.transpose(1, 0, 2).reshape(128, NB * 128))
    return [{"xp": host_planes(x[i]), "bands": bands} for i in range(NCORES)]


# revision 3
# speedup vs baseline: 1.0572x; 1.0143x over previous
"""Hamilton-Adams demosaic v2 for Trainium2 (8 cores, data-parallel).

Device computes 8 fp16 phase planes [R01,R10,R11,B00,B01,B10,G00,G11] from 8
host-prepared padded fp16 quarter planes; host handles plane prep (padding /
masking / dtype) and output interleave + identity phases (R00=x0, G01=x1,
G10=x2, B11=x3).

Layout: 7 row-strips of 128 lanes (lane p = qrow r0-2+p, ship lanes 2..125).
All compute is same-partition fp16 ops except cross-lane vertical taps, which
run on the PE as banded matmuls accumulating in PSUM.
"""
import sys
sys.path.insert(0, '/opt/trn_rl_repo')

from contextlib import ExitStack

import numpy as np

import concourse.bass as bass
import concourse.bacc as bacc
import concourse.tile as tile
from concourse import mybir
from concourse.bass_utils import run_bass_kernel_spmd

F16 = mybir.dt.float16
F32 = mybir.dt.float32
U8 = mybir.dt.uint8
U16 = mybir.dt.uint16
AL = mybir.AluOpType
AF = mybir.ActivationFunctionType

H = 768
W = 772            # padded plane width (col = qcol + 2)
NCORES = 8
STRIPS = [0, 124, 248, 372, 496, 620, 644]

# input plane slots (order in IT tile and xp array)
P00, P01, P10, P11, P00Z, P11Z, P00UZ, P11DZ = range(8)
# band matrix ids
BANDS = {}


def _band_defs():
    """band[k, m] = weight of moving lane k for output lane m."""
    def mk(taps):
        b = np.zeros((128, 128), np.float16)
        for dk, w in taps:
            for m in range(128):
                k = m + dk
                if 0 <= k < 128:
                    b[k, m] = w
        return b
    defs = {
        "tri_n05": [(-1, -0.5), (0, 1.0), (1, -0.5)],     # -0.5 * (1,-2,1)
        "duo_m": [(-1, 1.0), (0, 1.0)],
        "duo_p": [(0, 1.0), (1, 1.0)],
        "d5_m": [(-1, 1.0), (0, -1.0)],
        "d5_p": [(0, 1.0), (1, -1.0)],
        "I": [(0, 1.0)],
        "I05": [(0, 0.5)],
        "I_n05": [(0, -0.5)],
        "up1": [(1, 1.0)],
        "up1_n05": [(1, -0.5)],
        "dn1_n05": [(-1, -0.5)],
        "duo_p_05": [(0, 0.5), (1, 0.5)],
        "duo_m_05": [(-1, 0.5), (0, 0.5)],
        "duo_p_n025": [(0, -0.25), (1, -0.25)],
        "duo_m_n025": [(-1, -0.25), (0, -0.25)],
    }
    names = list(defs)
    arr = np.stack([mk(defs[n]) for n in names])  # [NB,128,128]
    return names, arr


BAND_NAMES, BAND_ARR = _band_defs()
NB = len(BAND_NAMES)


class Ctx:
    def __init__(self, nc, pools, bd):
        self.nc = nc
        self.it_pool, self.g_pool, self.ot_pool, self.tmp_pool, self.ps_pool = pools
        self.bd = bd  # band tile [128, NB*128]

    def band(self, name):
        i = BAND_NAMES.index(name)
        return self.bd[:, 128 * i : 128 * i + 128]

    def tmp(self):
        return self.tmp_pool.tile([128, H], F16, tag="tmp", name="tmp")

    def msk(self):
        return self.tmp_pool.tile([128, H], U16, tag="msk", name="msk")

    def psum(self, tag):
        # 1024 f32 = exactly 2 PSUM banks; only cols 0:768 are used, and the
        # matmul chunk split (0:512, 512:768) keeps each write within a bank.
        return self.ps_pool.tile([128, 1024], F32, tag=tag, name=tag)


def _mm_band(nc, dst, band, mov_tile, mov_off, start):
    """dst [128,768] psum += band.T @ mov (768 cols from mov_tile at mov_off)."""
    for c0, cw in ((0, 512), (512, 256)):
        nc.tensor.matmul(
            dst[:, c0 : c0 + cw],
            band,
            mov_tile[:, mov_off + c0 : mov_off + c0 + cw],
            start=start, stop=True,
        )


def pe_accum(cx, dst, terms, tag=None):
    """terms: list of (band_name, mov_tile, mov_off). Accumulates into dst psum."""
    nc = cx.nc
    for i, (bn, mt, mo) in enumerate(terms):
        _mm_band(nc, dst, cx.band(bn), mt, mo, start=(i == 0))


def _green_phases(cx, IT, G, phA, phB):
    """Emit both green phases interleaved so each engine always has ready work."""
    nc = cx.nc
    s = lambda k, dj=0: IT[:, 772 * k + 2 + dj : 772 * k + 2 + dj + H]

    pa, q3, pc, q5, ah, c0s, c4 = {}, {}, {}, {}, {}, {}, {}
    sh, c2, b2, q2, q4, clh, clv, m = {}, {}, {}, {}, {}, {}, {}, {}
    P = {0: phA, 1: phB}

    for i in (0, 1):
        pa[i] = cx.psum("psA")
        pe_accum(cx, pa[i], [("tri_n05", IT, 772 * P[i]["ctr"] + 2)])
    for i in (0, 1):
        q3[i] = cx.tmp(); nc.scalar.activation(q3[i][:], pa[i][:, :H], AF.Abs, 0.0, -2.0)
    for i in (0, 1):
        _mm_band(nc, pa[i], cx.band(P[i]["duo"]), IT, 772 * P[i]["vp"] + 2, start=False)
    for i in (0, 1):
        pc[i] = cx.psum("psB")
        pe_accum(cx, pc[i], [(P[i]["d5"], IT, 772 * P[i]["vp"] + 2)])
    for i in (0, 1):
        q5[i] = cx.tmp(); nc.scalar.activation(q5[i][:], pc[i][:, :H], AF.Abs)
    for i in (0, 1):
        ah[i] = cx.tmp(); nc.scalar.activation(ah[i][:], pa[i][:, :H], AF.Copy, 0.0, 0.5)
    for i in (0, 1):
        o0, o1 = P[i]["ho"]; hp = P[i]["hp"]
        c0s[i] = cx.tmp(); nc.gpsimd.tensor_tensor(c0s[i][:], s(hp, o0), s(hp, o1), AL.add)
    for i in (0, 1):
        o0, o1 = P[i]["ho"]; hp = P[i]["hp"]
        c4[i] = cx.tmp(); nc.gpsimd.tensor_tensor(c4[i][:], s(hp, o0), s(hp, o1), AL.subtract)
    for i in (0, 1):
        ctr = P[i]["ctr"]
        sh[i] = cx.tmp(); nc.vector.tensor_tensor(sh[i][:], s(ctr, -1), s(ctr, +1), AL.add)
    for i in (0, 1):
        c2[i] = cx.tmp(); nc.vector.scalar_tensor_tensor(c2[i][:], s(P[i]["ctr"]), -2.0, sh[i][:], AL.mult, AL.add)
    for i in (0, 1):
        q2[i] = cx.tmp(); nc.vector.tensor_scalar(q2[i][:].bitcast(U16), c2[i][:].bitcast(U16), 0x7FFF, None, AL.bitwise_and)
    for i in (0, 1):
        q4[i] = cx.tmp(); nc.vector.tensor_scalar(q4[i][:].bitcast(U16), c4[i][:].bitcast(U16), 0x7FFF, None, AL.bitwise_and)
    for i in (0, 1):
        b2[i] = cx.tmp(); nc.vector.scalar_tensor_tensor(b2[i][:], c2[i][:], -0.5, c0s[i][:], AL.mult, AL.add)
    for i in (0, 1):
        clh[i] = cx.tmp(); nc.vector.tensor_tensor(clh[i][:], q2[i][:], q4[i][:], AL.add)
    for i in (0, 1):
        clv[i] = cx.tmp(); nc.vector.tensor_tensor(clv[i][:], q3[i][:], q5[i][:], AL.add)
    for i in (0, 1):
        m[i] = cx.msk(); nc.vector.tensor_tensor(m[i][:], clh[i][:], clv[i][:], AL.is_gt)
    for i in (0, 1):
        gslot = P[i]["gslot"]
        gd = G[:, 772 * gslot + 2 : 772 * gslot + 2 + H]
        nc.vector.tensor_scalar(gd, b2[i][:], 0.5, None, AL.mult)
        nc.vector.copy_predicated(gd, m[i][:], ah[i][:])


def _chroma(cx, IT, G, OT, hvH, hvV, cbs):
    """Emit hv fields + chan blends with A/B interleaving."""
    nc = cx.nc
    s = lambda k, dj=0: IT[:, 772 * k + 2 + dj : 772 * k + 2 + dj + H]
    gs = lambda k, dj=0: G[:, 772 * k + 2 + dj : 772 * k + 2 + dj + H]

    # hv vertical via PE
    pv = {}
    for i, (xz, g, addp, duoX, duoG, oslot) in enumerate(hvV):
        pv[i] = cx.psum("psB")
        pe_accum(cx, pv[i], [
            (duoX, IT, 772 * xz + 2),
            (duoG, G, 772 * g + 2),
            ("I05", IT, 772 * addp + 2),
        ])
    for i, (xz, g, addp, duoX, duoG, oslot) in enumerate(hvV):
        nc.scalar.activation(OT[:, H * oslot : H * oslot + H], pv[i][:, :H], AF.Copy)

    # cb pool diffs
    dm, dn = {}, {}
    for i, ch in enumerate(cbs):
        cP, cN = ch["cP"], ch["cN"]
        dm[i] = cx.tmp(); nc.gpsimd.tensor_tensor(dm[i][:], s(ch["xs"], cP[1]), s(ch["xp"], cP[0]), AL.subtract)
    for i, ch in enumerate(cbs):
        cP, cN = ch["cP"], ch["cN"]
        dn[i] = cx.tmp(); nc.gpsimd.tensor_tensor(dn[i][:], s(ch["xs"], cN[1]), s(ch["xp"], cN[0]), AL.subtract)

    # cb PE banks (p then n), with scalar abs between accumulations
    pp, pn, qp, qn = {}, {}, {}, {}
    for i, ch in enumerate(cbs):
        pp[i] = cx.psum("psA")
        pe_accum(cx, pp[i], [
            ("I", G, 772 * ch["gnear"] + 2),
            (ch["gP0"][0], G, 772 * ch["gf"] + 2 + ch["gP0"][1]),
            (ch["gP1"][0], G, 772 * ch["gf"] + 2 + ch["gP1"][1]),
        ])
    for i, ch in enumerate(cbs):
        qp[i] = cx.tmp(); nc.scalar.activation(qp[i][:], pp[i][:, :H], AF.Abs, 0.0, -2.0)
    for i, ch in enumerate(cbs):
        _mm_band(nc, pp[i], cx.band("I"), IT, 772 * ch["xp"] + 2 + ch["cP"][0], start=False)
        _mm_band(nc, pp[i], cx.band("I"), IT, 772 * ch["xs"] + 2 + ch["cP"][1], start=False)

    # hv horizontal on DVE (independent work while PE/scalar chew on cb)
    t1, t2, u = {}, {}, {}
    for i, (xz, xo, g, go, addp, oslot) in enumerate(hvH):
        t1[i] = cx.tmp(); nc.vector.tensor_tensor(t1[i][:], s(xz, xo[0]), s(xz, xo[1]), AL.add)
    for i, (xz, xo, g, go, addp, oslot) in enumerate(hvH):
        t2[i] = cx.tmp(); nc.vector.tensor_tensor(t2[i][:], gs(g, go[0]), gs(g, go[1]), AL.add)
    for i, (xz, xo, g, go, addp, oslot) in enumerate(hvH):
        u[i] = cx.tmp(); nc.vector.scalar_tensor_tensor(u[i][:], t2[i][:], -0.5, t1[i][:], AL.mult, AL.add)
    for i, (xz, xo, g, go, addp, oslot) in enumerate(hvH):
        v = cx.tmp(); nc.vector.tensor_tensor(v[:], u[i][:], s(addp), AL.add)
        nc.vector.tensor_scalar(OT[:, H * oslot : H * oslot + H], v[:], 0.5, None, AL.mult)

    for i, ch in enumerate(cbs):
        pn[i] = cx.psum("psB")
        pe_accum(cx, pn[i], [
            ("I", G, 772 * ch["gnear"] + 2),
            (ch["gN0"][0], G, 772 * ch["gf"] + 2 + ch["gN0"][1]),
            (ch["gN1"][0], G, 772 * ch["gf"] + 2 + ch["gN1"][1]),
        ])
    for i, ch in enumerate(cbs):
        qn[i] = cx.tmp(); nc.scalar.activation(qn[i][:], pn[i][:, :H], AF.Abs, 0.0, -2.0)
    for i, ch in enumerate(cbs):
        _mm_band(nc, pn[i], cx.band("I"), IT, 772 * ch["xp"] + 2 + ch["cN"][0], start=False)
        _mm_band(nc, pn[i], cx.band("I"), IT, 772 * ch["xs"] + 2 + ch["cN"][1], start=False)

    # classifiers + select
    qdm, qdn, clp, cln, m2, cnh = {}, {}, {}, {}, {}, {}
    for i in range(len(cbs)):
        qdm[i] = cx.tmp(); nc.vector.tensor_scalar(qdm[i][:].bitcast(U16), dm[i][:].bitcast(U16), 0x7FFF, None, AL.bitwise_and)
    for i in range(len(cbs)):
        qdn[i] = cx.tmp(); nc.vector.tensor_scalar(qdn[i][:].bitcast(U16), dn[i][:].bitcast(U16), 0x7FFF, None, AL.bitwise_and)
    for i in range(len(cbs)):
        clp[i] = cx.tmp(); nc.vector.tensor_tensor(clp[i][:], qdm[i][:], qp[i][:], AL.add)
    for i in range(len(cbs)):
        cln[i] = cx.tmp(); nc.vector.tensor_tensor(cln[i][:], qdn[i][:], qn[i][:], AL.add)
    for i in range(len(cbs)):
        m2[i] = cx.msk(); nc.vector.tensor_tensor(m2[i][:], clp[i][:], cln[i][:], AL.is_gt)
    for i, ch in enumerate(cbs):
        osl = OT[:, H * ch["oslot"] : H * ch["oslot"] + H]
        nc.scalar.activation(osl, pp[i][:, :H], AF.Copy, 0.0, 0.5)
    for i, ch in enumerate(cbs):
        cnh[i] = cx.tmp(); nc.scalar.activation(cnh[i][:], pn[i][:, :H], AF.Copy, 0.0, 0.5)
    for i, ch in enumerate(cbs):
        osl = OT[:, H * ch["oslot"] : H * ch["oslot"] + H]
        nc.vector.copy_predicated(osl, m2[i][:], cnh[i][:])


# output plane order in device out tensor
OR01, OR10, OR11, OB00, OB01, OB10 = range(6)  # OT slots; G00, G11 from G tile
G00S, G11S = 0, 1


def _emit_green(cx, IT, G, xp_v, r0, first, last):
    nc = cx.nc
    it_v = IT[:].rearrange("p (k c) -> p k c", k=8)
    xr = xp_v.rearrange("k r c -> r k c")
    for k0 in range(0, 8, 2):
        nc.sync.dma_start(it_v[:, k0 : k0 + 2, :], xr[r0 : r0 + 128, k0 : k0 + 2, :])

    _green_phases(cx, IT, G,
                  dict(ctr=P00, hp=P01, ho=(-1, 0), vp=P10, duo="duo_m", d5="d5_m", gslot=G00S),
                  dict(ctr=P11, hp=P10, ho=(0, +1), vp=P01, duo="duo_p", d5="d5_p", gslot=G11S))

    # green halo cols: g00[:,770] = p01 col 767 ; g11[:,1] = p10 col 0
    nc.vector.tensor_copy(G[:, 770:771], IT[:, 772 * P01 + 769 : 772 * P01 + 770])
    nc.vector.tensor_copy(G[:, 772 + 1 : 772 + 2], IT[:, 772 * P10 + 2 : 772 * P10 + 3])
    if first:
        nc.sync.dma_start(G[1:2, 772:1544], IT[1:2, 772 * P01 : 772 * P01 + 772])
    if last:
        nc.sync.dma_start(G[126:127, 0:772], IT[126:127, 772 * P10 : 772 * P10 + 772])


def _emit_chroma(cx, IT, G, OT, out_v, r0, last):
    nc = cx.nc
    _chroma(cx, IT, G, OT,
            hvH=[(P00Z, (0, +1), G00S, (0, +1), P01, OR01),
                 (P11Z, (-1, 0), G11S, (-1, 0), P10, OB10)],
            hvV=[(P00Z, G00S, P10, "duo_p_05", "duo_p_n025", OR10),
                 (P11Z, G11S, P01, "duo_m_05", "duo_m_n025", OB01)],
            cbs=[dict(xp=P00Z, xs=P00UZ, cP=(0, +1), cN=(+1, 0), gf=G00S, gnear=G11S,
                      gP0=("I_n05", 0), gP1=("up1_n05", +1),
                      gN0=("I_n05", +1), gN1=("up1_n05", 0), oslot=OR11),
                 dict(xp=P11DZ, xs=P11Z, cP=(-1, 0), cN=(0, -1), gf=G11S, gnear=G00S,
                      gP0=("dn1_n05", -1), gP1=("I_n05", 0),
                      gN0=("dn1_n05", 0), gN1=("I_n05", -1), oslot=OB00)])

    if last:
        p0, pn_, row0 = 102, 24, 744
    else:
        p0, pn_, row0 = 2, 124, r0
    ot_v = OT[:].rearrange("p (k c) -> p k c", k=6)
    our = out_v.rearrange("k r c -> r k c")
    nc.sync.dma_start(our[row0 : row0 + pn_, 0:3, :], ot_v[p0 : p0 + pn_, 0:3, :])
    nc.sync.dma_start(our[row0 : row0 + pn_, 3:6, :], ot_v[p0 : p0 + pn_, 3:6, :])
    g_int = G[:].rearrange("p (k c) -> p k c", k=2)[:, :, 2 : 2 + H]
    nc.sync.dma_start(our[row0 : row0 + pn_, 6:8, :], g_int[p0 : p0 + pn_, :, :])


def build_nc():
    nc = bacc.Bacc("TRN2", target_bir_lowering=False, debug=False, num_devices=NCORES)
    xp_in = nc.declare_dram_parameter("xp", [8, W, W], F16, isOutput=False)
    bd_in = nc.declare_dram_parameter("bands", [128, NB * 128], F16, isOutput=False)
    out = nc.declare_dram_parameter("out", [8, H, H], F16, isOutput=True)

    with tile.TileContext(nc) as tc, ExitStack() as ctx:
        itp = ctx.enter_context(tc.tile_pool(name="it", bufs=3))
        gp = ctx.enter_context(tc.tile_pool(name="g", bufs=2))
        otp = ctx.enter_context(tc.tile_pool(name="ot", bufs=2))
        tmp = ctx.enter_context(tc.tile_pool(name="tmp", bufs=16))
        psp = ctx.enter_context(tc.tile_pool(name="ps", bufs=2, space="PSUM"))
        bdp = ctx.enter_context(tc.tile_pool(name="bd", bufs=1))

        bd = bdp.tile([128, NB * 128], F16, name="bands")
        nc.sync.dma_start(bd[:], bd_in[:])

        cx = Ctx(nc, (itp, gp, otp, tmp, psp), bd)
        nstrips = len(STRIPS)
        pend = None  # (IT, G, OT, r0, last) awaiting chroma emission
        for si, r0 in enumerate(STRIPS):
            IT = itp.tile([128, 8 * W], F16, tag="IT", name="IT")
            G = gp.tile([128, 2 * W], F16, tag="G", name="G")
            OT = otp.tile([128, 6 * H], F16, tag="OT", name="OT")
            _emit_green(cx, IT, G, xp_in[:], r0, si == 0, si == nstrips - 1)
            if pend is not None:
                _emit_chroma(cx, *pend)
            pend = (IT, G, OT, out[:], r0, si == nstrips - 1)
        _emit_chroma(cx, *pend)
    nc.compile()
    return nc


def host_planes(P):
    """P: [4,768,768] f32. Returns [8,772,772] fp16 padded planes."""
    M = np.empty((1536, 1536), np.float32)
    M[0::2, 0::2] = P[0]
    M[0::2, 1::2] = P[1]
    M[1::2, 0::2] = P[2]
    M[1::2, 1::2] = P[3]
    Mzr = np.zeros_like(M); Mzr[0::2, 0::2] = P[0]
    Mzb = np.zeros_like(M); Mzb[1::2, 1::2] = P[3]
    q = np.arange(-2, 770)

    def mk(src, a, b, dr=0):
        ri = np.clip(2 * (q + dr) + a, 0, 1535)
        ci = np.clip(2 * q + b, 0, 1535)
        return src[np.ix_(ri, ci)]

    pl = np.empty((8, 772, 772), np.float16)
    pl[P00] = mk(M, 0, 0)
    pl[P01] = mk(M, 0, 1)
    pl[P10] = mk(M, 1, 0)
    pl[P11] = mk(M, 1, 1)
    pl[P00Z] = mk(Mzr, 0, 0)
    pl[P11Z] = mk(Mzb, 1, 1)
    pl[P00UZ] = mk(Mzr, 0, 0, dr=+1)
    pl[P11DZ] = mk(Mzb, 1, 1, dr=-1)
    return pl


_NC_CACHE = None


def kernel(x: np.ndarray) -> np.ndarray:
    global _NC_CACHE
    if _NC_CACHE is None:
        _NC_CACHE = build_nc()
    x = np.ascontiguousarray(x, dtype=np.float32)
    bands = np.ascontiguousarray(BAND_ARR.transpose(1, 0, 2).reshape(128, NB * 128))
    in_maps = [{"xp": host_planes(x[i]), "bands": bands} for i in range(NCORES)]
    res = run_bass_kernel_spmd(_NC_CACHE, in_maps, list(range(NCORES)))
    out = np.empty((NCORES, 3, 1536, 1536), np.float32)
    for i in range(NCORES):
        d = res.results[i]["out"].astype(np.float32)
        out[i, 0, 0::2, 0::2] = x[i, 0]
        out[i, 0, 0::2, 1::2] = d[OR01]
        out[i, 0, 1::2, 0::2] = d[OR10]
        out[i, 0, 1::2, 1::2] = d[OR11]
        out[i, 1, 0::2, 0::2] = d[6]
        out[i, 1, 0::2, 1::2] = x[i, 1]
        out[i, 1, 1::2, 0::2] = x[i, 2]
        out[i, 1, 1::2, 1::2] = d[7]
        out[i, 2, 0::2, 0::2] = d[OB00]
        out[i, 2, 0::2, 1::2] = d[OB01]
        out[i, 2, 1::2, 0::2] = d[OB10]
        out[i, 2, 1::2, 1::2] = x[i, 3]
    return out


def make_trace_inmaps(x):
    """in_maps for a traced run (test harness helper)."""
    x = np.ascontiguousarray(x, dtype=np.float32)
    bands = np.ascontiguousarray(BAND_ARR.transpose(1, 0, 2).reshape(128, NB * 128))
    return [{"xp": host_planes(x[i]), "bands": bands} for i in range(NCORES)]
